# revision 14
# baseline (speedup 1.0000x reference)
"""BertSelfAttention on 8 TRN2 NeuronCores (Bass/Tile).

Sharding: tensor-parallel over heads. Core c computes heads 2c, 2c+1
(output dims 128c : 128c+128). Each core receives the full hidden states
(pre-transposed and cast to bf16 on the host) plus its slice of the
Q/K/V projection weights, and produces its [B, 2, 65, S] slice of
UNNORMALIZED context (transposed, [d, q] layout) plus the softmax row
sums l (row 64 of each head's 65-row block); the host divides by l,
transposes to [B, S, dh] and concatenates along the feature axis.

Per-core pipeline (B=4, S=2048, H=1024, NH=16, HD=64; 2 heads/core):
  1. proj:  QT,KT [128, S] (head dim on partitions, heads stacked
            h0=0:64 / h1=64:128) and V [S, 128] (seq on partitions) via
            PE matmuls over 8 contraction chunks.
  2. attn:  for each 512-wide query chunk:
              for each 128-wide key block:
                ST[k,q] = KT^T @ QT  (two heads = two row-tiled matmuls
                                      on PE array halves, contraction 64)
                P = exp(ST):  ~9/16 blocks on ScalarE (ACTIVATE Exp,
                              scale ln2*2^-23), ~7/16 on the Vector
                              engine via a custom exp2-bit-trick op
                              (EXP2_BITS_ANT): computes the bfloat16 bit
                              pattern of 2^t as an fp32 VALUE, writes it
                              int16-converted; the context matmul reads
                              it back bitcast as bfloat16.
                OT[d,q] += [V | 1]^T matmul: lhsT=[V,ones] (M=65)
                           accumulating both context and row-sums l.
              evacuate OT psum -> sbuf -> HBM (still [d, q]; the host
              normalizes by l and transposes).
The bv bias is folded in on the host (rows of softmax sum to one, so
ctx(V + bv) = ctx(V) + bv exactly).
"""

import numpy as np
import ml_dtypes

import concourse.bass as bass
import concourse.mybir as mybir
import concourse.tile as tile
from concourse import bass_utils

B, S, H, NH, HD = 4, 2048, 1024, 16, 64
N_CORES = 8
DH = H // N_CORES          # 128 output dims per core (2 heads)
P = 128
QC = 512                   # query chunk (psum bank width in fp32)
NQC = S // QC              # 4
NKB = S // P               # 16 key blocks
NHC = H // P               # 8 contraction chunks for the projections
BF16 = mybir.dt.bfloat16
F32 = mybir.dt.float32



def _split_multi_waits(nc):
    # walrus in this container accepts at most ONE sync wait per
    # instruction; hoist extra waits onto preceding same-engine NOPs.
    n = 0
    for bb in nc.m.functions[0].blocks:
        new_insts = []
        for inst in bb.instructions:
            si = inst.sync_info
            if si is not None and si.on_wait:
                waits = list(si.on_wait)
                for w in waits[:-1]:
                    n += 1
                    new_insts.append(
                        mybir.InstNoOp(
                            name=f"waitsplit_{n}",
                            engine=inst.engine,
                            bass_nofuse=True,
                            sync_info=mybir.SyncInfo(on_wait=[w], on_update=[]),
                        )
                    )
                si.on_wait = waits[-1:]
            new_insts.append(inst)
        bb.instructions[:] = new_insts


def build_bass(reps=1):
    nc = bass.Bass("TRN2", target_bir_lowering=False, debug=False)
    xt = nc.dram_tensor("xt", [B, H, S], BF16, kind="ExternalInput").ap()
    wqt = nc.dram_tensor("wqt", [P, NHC * DH], BF16, kind="ExternalInput").ap()
    wkt = nc.dram_tensor("wkt", [P, NHC * DH], BF16, kind="ExternalInput").ap()
    wvt = nc.dram_tensor("wvt", [P, NHC * DH], BF16, kind="ExternalInput").ap()
    bqv = nc.dram_tensor("bqv", [DH], F32, kind="ExternalInput").ap()
    bkv = nc.dram_tensor("bkv", [DH], F32, kind="ExternalInput").ap()
    mask = nc.dram_tensor("mask", [B, S], F32, kind="ExternalInput").ap()
    # rows 0:64 = ctx_h (unnormalized, [d, q]); row 64 = l_h
    out = nc.dram_tensor("out", [B, 2, HD + 1, S], F32, kind="ExternalOutput").ap()

    with tile.TileContext(nc) as tc:
        from contextlib import ExitStack

        with ExitStack() as ctx:
            consts = ctx.enter_context(tc.tile_pool(name="consts", bufs=1))
            xt_pool = ctx.enter_context(tc.tile_pool(name="xt", bufs=2))
            qkt_pool = ctx.enter_context(tc.tile_pool(name="qkt", bufs=2))
            von_pool = ctx.enter_context(tc.tile_pool(name="von", bufs=2))
            ex_pool = ctx.enter_context(tc.tile_pool(name="ex", bufs=3))
            osb_pool = ctx.enter_context(tc.tile_pool(name="osb", bufs=2))
            mask_pool = ctx.enter_context(tc.tile_pool(name="maskp", bufs=2))
            ps_misc = ctx.enter_context(tc.tile_pool(name="ps_misc", bufs=2, space="PSUM"))
            ps_st = ctx.enter_context(tc.tile_pool(name="ps_st", bufs=2, space="PSUM"))
            ps_ot = ctx.enter_context(tc.tile_pool(name="ps_ot", bufs=2, space="PSUM"))

            # constants
            wq_sb = consts.tile([P, NHC, DH], BF16, name="wq_sb")
            wk_sb = consts.tile([P, NHC, DH], BF16, name="wk_sb")
            wv_sb = consts.tile([P, NHC, DH], BF16, name="wv_sb")
            # weights pre-arranged on the host to [p, hc*d]: contiguous
            # 2KB per partition per DMA, so they land in ~2us and the PE
            # warm-up + first projections start immediately.
            nc.sync.dma_start(wq_sb[:], wqt.rearrange("p (hc d) -> p hc d", hc=NHC))
            nc.sync.dma_start(wk_sb[:], wkt.rearrange("p (hc d) -> p hc d", hc=NHC))
            nc.sync.dma_start(wv_sb[:], wvt.rearrange("p (hc d) -> p hc d", hc=NHC))
            bq_sb = consts.tile([P, 1], F32, name="bq_sb")
            bk_sb = consts.tile([P, 1], F32, name="bk_sb")
            nc.sync.dma_start(bq_sb[:], bqv[:, None])
            nc.sync.dma_start(bk_sb[:], bkv[:, None])

            # PE warm-up during the initial xt DMA: ~4us of dummy matmuls
            # flips the HAM clock gate to 8/8 before the real stream.
            warm = ps_misc.tile([P, P], F32, name="warm", tag="misc")
            for _ in range(48):
                nc.tensor.matmul(
                    warm[:], lhsT=wq_sb[:, 0, :], rhs=wq_sb[:, 0, :],
                    start=True, stop=True,
                )

            def start_b(b):
                """Allocate per-batch tiles, issue input DMAs, and build the
                list of projection work units (each ~8 matmuls + 1 copy)."""
                st = {}
                st["xt"] = xt_pool.tile([P, NHC, S], BF16, name="xt_b", tag="xt_b")
                xr = xt[b].rearrange("(hc p) s -> p hc s", p=P)
                for cb in range(NHC):
                    # column-block DMAs (all hc chunks per 256-col block):
                    # a projection unit for columns [256c, 256c+256) only
                    # waits for block c, so compute starts after 1/8 of
                    # the 4MB slice instead of all of it.
                    csl = slice(cb * 2 * P, (cb + 1) * 2 * P)
                    nc.sync.dma_start(st["xt"][:, :, csl], xr[:, :, csl])
                st["mask"] = mask_pool.tile([P, NKB], F32, name="mask_b", tag="mask_b")
                nc.sync.dma_start(
                    st["mask"][:], mask[b].rearrange("(kb p) -> p kb", p=P)
                )
                st["qt"] = qkt_pool.tile([P, S], BF16, name="qt", tag="qt")
                st["kt"] = qkt_pool.tile([P, S], BF16, name="kt", tag="kt")
                st["von"] = von_pool.tile(
                    [P, NKB, 2 * (HD + 1)], BF16, name="von", tag="von"
                )
                nc.vector.memset(st["von"][:, :, HD:HD + 1], 1.0)
                nc.vector.memset(st["von"][:, :, 2 * HD + 1:2 * HD + 2], 1.0)
                # Ordered so attention can start after the first 6 units
                # (kt cols 0:256, qt chunk 0, von blocks 0-2); the rest is
                # injected 2-per-kb into the first attention qc's k-loop.
                st["units"] = (
                    [("pk", 0), ("pq", 0), ("pq", 1), ("pv", 0), ("pv", 1), ("pv", 2)]
                    + [("pk", 1), ("pv", 3), ("pv", 4), ("pk", 2), ("pv", 5),
                       ("pv", 6), ("pk", 3), ("pv", 7), ("pv", 8), ("pk", 4),
                       ("pv", 9), ("pv", 10), ("pk", 5), ("pv", 11), ("pv", 12),
                       ("pk", 6), ("pv", 13), ("pv", 14), ("pk", 7), ("pv", 15),
                       ("pq", 2), ("pq", 3), ("pq", 4), ("pq", 5), ("pq", 6),
                       ("pq", 7)]
                )
                return st

            HQ = QC // 2

            def emit_unit(st, unit):
                kind, idx = unit
                if kind in ("pq", "pk"):
                    w_sb = wq_sb if kind == "pq" else wk_sb
                    b_sb = bq_sb if kind == "pq" else bk_sb
                    dest = st["qt"] if kind == "pq" else st["kt"]
                    pp = ps_misc.tile([P, HQ], F32, name=kind, tag="misc")
                    for h in range(NHC):
                        nc.tensor.matmul(
                            pp[:],
                            lhsT=w_sb[:, h, :],
                            rhs=st["xt"][:, h, idx * HQ:(idx + 1) * HQ],
                            start=(h == 0),
                            stop=(h == NHC - 1),
                        )
                    nc.vector.tensor_tensor(
                        dest[:, idx * HQ:(idx + 1) * HQ],
                        pp[:],
                        b_sb[:].to_broadcast((P, HQ)),
                        mybir.AluOpType.add,
                    )
                else:  # pv: V block idx in [s, d] layout
                    pv = ps_misc.tile([P, P], F32, name="pv", tag="misc")
                    for h in range(NHC):
                        nc.tensor.matmul(
                            pv[:],
                            lhsT=st["xt"][:, h, idx * P:(idx + 1) * P],
                            rhs=wv_sb[:, h, :],
                            start=(h == 0),
                            stop=(h == NHC - 1),
                        )
                    nc.vector.tensor_copy(st["von"][:, idx, 0:HD], pv[:, 0:HD])
                    nc.vector.tensor_copy(
                        st["von"][:, idx, HD + 1:2 * HD + 1], pv[:, HD:2 * HD]
                    )

            seq = [b for _ in range(reps) for b in range(B)]
            state = {}
            # prologue for the first batch: just enough projections to
            # start attention; the rest injects into the early slots.
            state[0] = start_b(seq[0])
            for u in state[0]["units"][:6]:
                emit_unit(state[0], u)
            own_pending = list(state[0]["units"][6:])

            # Flat (batch, qc, kb) item stream, software-pipelined so the
            # ScalarE exp stream never waits on a just-issued matmul:
            #   slot t emits  scores(t+1) -> PE,  exp(t) -> ScalarE,
            #                 ctx(t-1) -> PE,  projection fill -> PE.
            # Tile keeps per-engine program order, so this ordering is what
            # the hardware executes.
            items = [
                (pos, qc, kb)
                for pos in range(len(seq))
                for qc in range(NQC)
                for kb in range(NKB)
            ]
            T = len(items)
            irec = {}
            otrec = {}
            inj = {}

            def emit_scores(t):
                pos, qc, kb = items[t]
                st_ = state[pos]
                qsl = slice(qc * QC, (qc + 1) * QC)
                stp = ps_st.tile([P, 2, QC], F32, name="stp")
                nc.tensor.matmul(
                    stp[:, 0, :],
                    lhsT=st_["kt"][0:HD, kb * P:(kb + 1) * P],
                    rhs=st_["qt"][0:HD, qsl],
                    start=True,
                    stop=True,
                )
                nc.tensor.matmul(
                    stp[:, 1, :],
                    lhsT=st_["kt"][HD:2 * HD, kb * P:(kb + 1) * P],
                    rhs=st_["qt"][HD:2 * HD, qsl],
                    start=True,
                    stop=True,
                )
                irec[t] = {"stp": stp}

            def emit_act(t):
                pos, qc, kb = items[t]
                st_ = state[pos]
                ex = ex_pool.tile([P, 2, QC], BF16, name="ex")
                nc.scalar.activation(
                    ex[:],
                    irec[t]["stp"][:],
                    mybir.ActivationFunctionType.Exp,
                    bias=st_["mask"][:, kb:kb + 1],
                    scale=1.0 / np.sqrt(HD),
                )
                irec[t]["ex"] = ex

            def emit_ctx(t):
                pos, qc, kb = items[t]
                st_ = state[pos]
                von = st_["von"]
                if kb == 0:
                    ot0 = ps_ot.tile([P, QC], F32, name="ot0", tag="ot")
                    ot1 = ps_ot.tile([P, QC], F32, name="ot1", tag="ot")
                    otrec[(pos, qc)] = (ot0, ot1)
                ot0, ot1 = otrec[(pos, qc)]
                ex = irec[t]["ex"]
                nc.tensor.matmul(
                    ot0[0:HD + 1, :],
                    lhsT=von[:, kb, 0:HD + 1],
                    rhs=ex[:, 0, :],
                    start=(kb == 0),
                    stop=(kb == NKB - 1),
                )
                nc.tensor.matmul(
                    ot1[0:HD + 1, :],
                    lhsT=von[:, kb, HD + 1:2 * HD + 2],
                    rhs=ex[:, 1, :],
                    start=(kb == 0),
                    stop=(kb == NKB - 1),
                )
                del irec[t]
                if kb == NKB - 1:
                    # evacuate [d, q] context + l rows to sbuf, then HBM;
                    # the host divides by l and transposes.
                    qsl = slice(qc * QC, (qc + 1) * QC)
                    osb = osb_pool.tile([HD + 1, 2, QC], F32, name="osb")
                    nc.vector.tensor_copy(osb[:, 0, :], ot0[0:HD + 1, :])
                    nc.vector.tensor_copy(osb[:, 1, :], ot1[0:HD + 1, :])
                    nc.sync.dma_start(
                        out[seq[pos]].rearrange("h d s -> d h s")[:, :, qsl],
                        osb[:],
                    )
                    del otrec[(pos, qc)]

            emit_scores(0)
            for t in range(T):
                pos, qc, kb = items[t]
                if qc == 0 and kb == 0:
                    if pos + 1 < len(seq):
                        state[pos + 1] = start_b(seq[pos + 1])
                    state.pop(pos - 2, None)
                emit_act(t)
                # PE slot order: ctx pair first (its von weight-load hides
                # behind the previous slot's full-mode matmuls), projection
                # fill, then the row-tiled scores pair last (one tiling-mode
                # switch right before it, drained while ACT(t) still runs).
                if t > 0:
                    emit_ctx(t - 1)
                if own_pending:
                    for u in own_pending[:4]:
                        emit_unit(state[0], u)
                    del own_pending[:4]
                elif pos + 1 < len(seq):
                    units = state[pos + 1]["units"]
                    it = qc * NKB + kb
                    target = min(
                        len(units),
                        it * len(units) // (NQC * NKB - 16) + 1,
                    )
                    cur = inj.get(pos + 1, 0)
                    if cur < target:
                        emit_unit(state[pos + 1], units[cur])
                        inj[pos + 1] = cur + 1
                if t + 1 < T:
                    emit_scores(t + 1)
            emit_ctx(T - 1)
    _split_multi_waits(nc)
    return nc


def host_prep(hidden_states, attention_mask, Wq, bq, Wk, bk, Wv, bv):
    xt_np = np.ascontiguousarray(
        np.asarray(hidden_states).transpose(0, 2, 1)
    ).astype(ml_dtypes.bfloat16)
    mask_np = np.ascontiguousarray(
        np.asarray(attention_mask).reshape(B, S)
    ).astype(np.float32)
    def wprep(W, dsl):
        # [H, DH] slice -> transposed [H, DH] -> [p, hc*d] so the device
        # DMA is contiguous per partition
        wt = np.asarray(W)[dsl, :].T.astype(ml_dtypes.bfloat16)   # [H, DH]
        return np.ascontiguousarray(
            wt.reshape(NHC, P, DH).transpose(1, 0, 2).reshape(P, NHC * DH)
        )

    in_maps = []
    for c in range(N_CORES):
        dsl = slice(c * DH, (c + 1) * DH)
        in_maps.append(
            {
                "xt": xt_np,
                "wqt": wprep(Wq, dsl),
                "wkt": wprep(Wk, dsl),
                "wvt": wprep(Wv, dsl),
                "bqv": np.ascontiguousarray(np.asarray(bq)[dsl]).astype(np.float32),
                "bkv": np.ascontiguousarray(np.asarray(bk)[dsl]).astype(np.float32),
                "mask": mask_np,
            }
        )
    return in_maps


def gather(results, bv):
    out = np.empty((B, S, H), np.float32)
    for c in range(N_CORES):
        r = results[c]["out"]  # [B, 2, HD+1, S]
        ctx = r[:, :, 0:HD, :] / r[:, :, HD:HD + 1, :]   # normalize by l
        # [B, 2, HD, S] -> [B, S, 2*HD]
        out[:, :, c * DH:(c + 1) * DH] = (
            ctx.transpose(0, 3, 1, 2).reshape(B, S, DH)
        )
    # bv folded on the host: softmax rows sum to 1, so ctx(V+bv)=ctx(V)+bv
    out += np.asarray(bv).astype(np.float32)[None, None, :]
    return out


def make_runner(nc, in_maps):
    """Build a reusable jitted 8-core runner for `nc` (mirrors
    bass2jax.run_bass_via_pjrt's multi-core path, but keeps the jitted
    callable so repeated executions don't re-lower)."""
    import jax
    from jax.sharding import Mesh, NamedSharding, PartitionSpec
    from jax.experimental.shard_map import shard_map
    from concourse import bass2jax

    bass2jax.install_neuronx_cc_hook()
    partition_name = nc.partition_id_tensor.name if nc.partition_id_tensor else None
    in_names, out_names, out_avals, zero_outs = [], [], [], []
    for alloc in nc.m.functions[0].allocations:
        if not isinstance(alloc, mybir.MemoryLocationSet):
            continue
        name = alloc.memorylocations[0].name
        if alloc.kind == "ExternalInput":
            if name != partition_name:
                in_names.append(name)
        elif alloc.kind == "ExternalOutput":
            out_names.append(name)
            shape = tuple(alloc.tensor_shape)
            dtype = mybir.dt.np(alloc.dtype)
            out_avals.append(jax.core.ShapedArray(shape, dtype))
            zero_outs.append(np.zeros(shape, dtype))
    n_params = len(in_names)
    n_outs = len(out_avals)
    all_in = list(in_names) + list(out_names)
    if partition_name is not None:
        all_in.append(partition_name)

    def _body(*args):
        operands = list(args)
        if partition_name is not None:
            operands.append(bass2jax.partition_id_tensor())
        outs = bass2jax._bass_exec_p.bind(
            *operands,
            out_avals=tuple(out_avals),
            in_names=tuple(all_in),
            out_names=tuple(out_names),
            lowering_input_output_aliases=(),
            sim_require_finite=True,
            sim_require_nnan=True,
            nc=nc,
        )
        return tuple(outs)

    devices = jax.devices()[:N_CORES]
    mesh = Mesh(np.asarray(devices), ("core",))
    sharded = jax.jit(
        shard_map(
            _body,
            mesh=mesh,
            in_specs=(PartitionSpec("core"),) * (n_params + n_outs),
            out_specs=(PartitionSpec("core"),) * n_outs,
            check_rep=False,
        ),
        keep_unused=True,
    )
    per_core = [[np.asarray(m[name]) for name in in_names[:n_params]] for m in in_maps]
    concat_in = [
        np.concatenate([per_core[c][i] for c in range(N_CORES)], axis=0)
        for i in range(n_params)
    ]
    concat_zeros = [
        np.zeros((N_CORES * z.shape[0], *z.shape[1:]), z.dtype) for z in zero_outs
    ]
    sh = NamedSharding(mesh, PartitionSpec("core"))
    args_dev = [jax.device_put(a, sh) for a in concat_in] + [
        jax.device_put(a, sh) for a in concat_zeros
    ]

    def run():
        import jax as _jax

        outs = sharded(*args_dev)
        _jax.block_until_ready(outs)
        return [
            {
                name: np.asarray(outs[i]).reshape(N_CORES, *out_avals[i].shape)[c]
                for i, name in enumerate(out_names)
            }
            for c in range(N_CORES)
        ]

    def run_nofetch():
        import jax as _jax

        outs = sharded(*args_dev)
        _jax.block_until_ready(outs)

    run.nofetch = run_nofetch
    return run


def kernel(hidden_states, attention_mask, Wq, bq, Wk, bk, Wv, bv):
    in_maps = host_prep(hidden_states, attention_mask, Wq, bq, Wk, bk, Wv, bv)
    nc = build_bass()
    res = bass_utils.run_bass_kernel_spmd(nc, in_maps, core_ids=list(range(N_CORES)))
    return gather(res.results, bv)


# revision 15
# speedup vs baseline: 1.0247x; 1.0247x over previous
"""BertSelfAttention on 8 TRN2 NeuronCores (Bass/Tile).

Sharding: tensor-parallel over heads. Core c computes heads 2c, 2c+1
(output dims 128c : 128c+128). Each core receives the full hidden states
(pre-transposed and cast to bf16 on the host) plus its slice of the
Q/K/V projection weights, and produces its [B, 2, 65, S] slice of
UNNORMALIZED context (transposed, [d, q] layout) plus the softmax row
sums l (row 64 of each head's 65-row block); the host divides by l,
transposes to [B, S, dh] and concatenates along the feature axis.

Per-core pipeline (B=4, S=2048, H=1024, NH=16, HD=64; 2 heads/core):
  1. proj:  QT,KT [128, S] (head dim on partitions, heads stacked
            h0=0:64 / h1=64:128) and V [S, 128] (seq on partitions) via
            PE matmuls over 8 contraction chunks.
  2. attn:  for each 512-wide query chunk:
              for each 128-wide key block:
                ST[k,q] = KT^T @ QT  (two heads = two row-tiled matmuls
                                      on PE array halves, contraction 64)
                P = exp(ST):  ~9/16 blocks on ScalarE (ACTIVATE Exp,
                              scale ln2*2^-23), ~7/16 on the Vector
                              engine via a custom exp2-bit-trick op
                              (EXP2_BITS_ANT): computes the bfloat16 bit
                              pattern of 2^t as an fp32 VALUE, writes it
                              int16-converted; the context matmul reads
                              it back bitcast as bfloat16.
                OT[d,q] += [V | 1]^T matmul: lhsT=[V,ones] (M=65)
                           accumulating both context and row-sums l.
              evacuate OT psum -> sbuf -> HBM (still [d, q]; the host
              normalizes by l and transposes).
The bv bias is folded in on the host (rows of softmax sum to one, so
ctx(V + bv) = ctx(V) + bv exactly).
"""

import numpy as np
import ml_dtypes

import concourse.bass as bass
import concourse.mybir as mybir
import concourse.tile as tile
from concourse import bass_utils

B, S, H, NH, HD = 4, 2048, 1024, 16, 64
N_CORES = 8
DH = H // N_CORES          # 128 output dims per core (2 heads)
P = 128
QC = 512                   # query chunk (psum bank width in fp32)
NQC = S // QC              # 4
NKB = S // P               # 16 key blocks
NHC = H // P               # 8 contraction chunks for the projections
BF16 = mybir.dt.bfloat16
F32 = mybir.dt.float32



def _split_multi_waits(nc):
    # walrus in this container accepts at most ONE sync wait per
    # instruction; hoist extra waits onto preceding same-engine NOPs.
    n = 0
    for bb in nc.m.functions[0].blocks:
        new_insts = []
        for inst in bb.instructions:
            si = inst.sync_info
            if si is not None and si.on_wait:
                waits = list(si.on_wait)
                for w in waits[:-1]:
                    n += 1
                    new_insts.append(
                        mybir.InstNoOp(
                            name=f"waitsplit_{n}",
                            engine=inst.engine,
                            bass_nofuse=True,
                            sync_info=mybir.SyncInfo(on_wait=[w], on_update=[]),
                        )
                    )
                si.on_wait = waits[-1:]
            new_insts.append(inst)
        bb.instructions[:] = new_insts


def build_bass(reps=1):
    nc = bass.Bass("TRN2", target_bir_lowering=False, debug=False)
    xt = nc.dram_tensor("xt", [B, H, S], BF16, kind="ExternalInput").ap()
    wqt = nc.dram_tensor("wqt", [P, NHC * DH], BF16, kind="ExternalInput").ap()
    wkt = nc.dram_tensor("wkt", [P, NHC * DH], BF16, kind="ExternalInput").ap()
    wvt = nc.dram_tensor("wvt", [P, NHC * DH], BF16, kind="ExternalInput").ap()
    bqv = nc.dram_tensor("bqv", [DH], F32, kind="ExternalInput").ap()
    bkv = nc.dram_tensor("bkv", [DH], F32, kind="ExternalInput").ap()
    mask = nc.dram_tensor("mask", [B, S], F32, kind="ExternalInput").ap()
    # rows 0:64 = ctx_h (unnormalized, [d, q]); row 64 = l_h
    out = nc.dram_tensor("out", [B, 2, HD + 1, S], F32, kind="ExternalOutput").ap()

    with tile.TileContext(nc) as tc:
        from contextlib import ExitStack

        with ExitStack() as ctx:
            consts = ctx.enter_context(tc.tile_pool(name="consts", bufs=1))
            xt_pool = ctx.enter_context(tc.tile_pool(name="xt", bufs=2))
            qkt_pool = ctx.enter_context(tc.tile_pool(name="qkt", bufs=2))
            von_pool = ctx.enter_context(tc.tile_pool(name="von", bufs=2))
            ex_pool = ctx.enter_context(tc.tile_pool(name="ex", bufs=3))
            osb_pool = ctx.enter_context(tc.tile_pool(name="osb", bufs=2))
            mask_pool = ctx.enter_context(tc.tile_pool(name="maskp", bufs=2))
            ps_misc = ctx.enter_context(tc.tile_pool(name="ps_misc", bufs=2, space="PSUM"))
            ps_st = ctx.enter_context(tc.tile_pool(name="ps_st", bufs=2, space="PSUM"))
            ps_ot = ctx.enter_context(tc.tile_pool(name="ps_ot", bufs=2, space="PSUM"))

            # constants
            wq_sb = consts.tile([P, NHC, DH], BF16, name="wq_sb")
            wk_sb = consts.tile([P, NHC, DH], BF16, name="wk_sb")
            wv_sb = consts.tile([P, NHC, DH], BF16, name="wv_sb")
            # weights pre-arranged on the host to [p, hc*d]: contiguous
            # 2KB per partition per DMA, so they land in ~2us and the PE
            # warm-up + first projections start immediately.
            nc.sync.dma_start(wq_sb[:], wqt.rearrange("p (hc d) -> p hc d", hc=NHC))
            nc.sync.dma_start(wk_sb[:], wkt.rearrange("p (hc d) -> p hc d", hc=NHC))
            nc.sync.dma_start(wv_sb[:], wvt.rearrange("p (hc d) -> p hc d", hc=NHC))
            bq_sb = consts.tile([P, 1], F32, name="bq_sb")
            bk_sb = consts.tile([P, 1], F32, name="bk_sb")
            nc.sync.dma_start(bq_sb[:], bqv[:, None])
            nc.sync.dma_start(bk_sb[:], bkv[:, None])

            # PE warm-up during the initial xt DMA: ~4us of dummy matmuls
            # flips the HAM clock gate to 8/8 before the real stream.
            warm = ps_misc.tile([P, P], F32, name="warm", tag="misc")
            for _ in range(48):
                nc.tensor.matmul(
                    warm[:], lhsT=wq_sb[:, 0, :], rhs=wq_sb[:, 0, :],
                    start=True, stop=True,
                )

            def start_b(b):
                """Allocate per-batch tiles, issue input DMAs, and build the
                list of projection work units (each ~8 matmuls + 1 copy)."""
                st = {}
                st["xt"] = xt_pool.tile([P, NHC, S], BF16, name="xt_b", tag="xt_b")
                xr = xt[b].rearrange("(hc p) s -> p hc s", p=P)
                for cb in range(NHC):
                    # column-block DMAs (all hc chunks per 256-col block):
                    # a projection unit for columns [256c, 256c+256) only
                    # waits for block c, so compute starts after 1/8 of
                    # the 4MB slice instead of all of it.
                    csl = slice(cb * 2 * P, (cb + 1) * 2 * P)
                    nc.sync.dma_start(st["xt"][:, :, csl], xr[:, :, csl])
                st["mask"] = mask_pool.tile([P, NKB], F32, name="mask_b", tag="mask_b")
                nc.sync.dma_start(
                    st["mask"][:], mask[b].rearrange("(kb p) -> p kb", p=P)
                )
                st["qt"] = qkt_pool.tile([P, S], BF16, name="qt", tag="qt")
                st["kt"] = qkt_pool.tile([P, S], BF16, name="kt", tag="kt")
                st["von"] = von_pool.tile(
                    [P, NKB, 2 * (HD + 1)], BF16, name="von", tag="von"
                )
                nc.vector.memset(st["von"][:, :, HD:HD + 1], 1.0)
                nc.vector.memset(st["von"][:, :, 2 * HD + 1:2 * HD + 2], 1.0)
                # Ordered so attention can start after the first 6 units
                # (kt cols 0:256, qt chunk 0, von blocks 0-2); the rest is
                # injected 2-per-kb into the first attention qc's k-loop.
                st["units"] = (
                    [("pk", 0), ("pq", 0), ("pq", 1), ("pv", 0), ("pv", 1), ("pv", 2)]
                    + [("pk", 1), ("pv", 3), ("pv", 4), ("pk", 2), ("pv", 5),
                       ("pv", 6), ("pk", 3), ("pv", 7), ("pv", 8), ("pk", 4),
                       ("pv", 9), ("pv", 10), ("pk", 5), ("pv", 11), ("pv", 12),
                       ("pk", 6), ("pv", 13), ("pv", 14), ("pk", 7), ("pv", 15),
                       ("pq", 2), ("pq", 3), ("pq", 4), ("pq", 5), ("pq", 6),
                       ("pq", 7)]
                )
                return st

            HQ = QC // 2

            def emit_unit(st, unit):
                kind, idx = unit
                if kind in ("pq", "pk"):
                    w_sb = wq_sb if kind == "pq" else wk_sb
                    b_sb = bq_sb if kind == "pq" else bk_sb
                    dest = st["qt"] if kind == "pq" else st["kt"]
                    pp = ps_misc.tile([P, HQ], F32, name=kind, tag="misc")
                    for h in range(NHC):
                        nc.tensor.matmul(
                            pp[:],
                            lhsT=w_sb[:, h, :],
                            rhs=st["xt"][:, h, idx * HQ:(idx + 1) * HQ],
                            start=(h == 0),
                            stop=(h == NHC - 1),
                        )
                    nc.vector.tensor_tensor(
                        dest[:, idx * HQ:(idx + 1) * HQ],
                        pp[:],
                        b_sb[:].to_broadcast((P, HQ)),
                        mybir.AluOpType.add,
                    )
                else:  # pv: V block idx in [s, d] layout
                    pv = ps_misc.tile([P, P], F32, name="pv", tag="misc")
                    for h in range(NHC):
                        nc.tensor.matmul(
                            pv[:],
                            lhsT=st["xt"][:, h, idx * P:(idx + 1) * P],
                            rhs=wv_sb[:, h, :],
                            start=(h == 0),
                            stop=(h == NHC - 1),
                        )
                    nc.vector.tensor_copy(st["von"][:, idx, 0:HD], pv[:, 0:HD])
                    nc.vector.tensor_copy(
                        st["von"][:, idx, HD + 1:2 * HD + 1], pv[:, HD:2 * HD]
                    )

            seq = [b for _ in range(reps) for b in range(B)]
            state = {}
            # prologue for the first batch: just enough projections to
            # start attention; the rest injects into the early slots.
            state[0] = start_b(seq[0])
            for u in state[0]["units"][:6]:
                emit_unit(state[0], u)
            own_pending = list(state[0]["units"][6:])

            # Flat (batch, qc, kb) item stream, software-pipelined so the
            # ScalarE exp stream never waits on a just-issued matmul:
            #   slot t emits  scores(t+1) -> PE,  exp(t) -> ScalarE,
            #                 ctx(t-1) -> PE,  projection fill -> PE.
            # Tile keeps per-engine program order, so this ordering is what
            # the hardware executes.
            items = [
                (pos, qc, kb)
                for pos in range(len(seq))
                for qc in range(NQC)
                for kb in range(NKB)
            ]
            T = len(items)
            irec = {}
            otrec = {}
            inj = {}

            def emit_scores(t):
                pos, qc, kb = items[t]
                st_ = state[pos]
                qsl = slice(qc * QC, (qc + 1) * QC)
                stp = ps_st.tile([P, 2, QC], F32, name="stp")
                nc.tensor.matmul(
                    stp[:, 0, :],
                    lhsT=st_["kt"][0:HD, kb * P:(kb + 1) * P],
                    rhs=st_["qt"][0:HD, qsl],
                    start=True,
                    stop=True,
                )
                nc.tensor.matmul(
                    stp[:, 1, :],
                    lhsT=st_["kt"][HD:2 * HD, kb * P:(kb + 1) * P],
                    rhs=st_["qt"][HD:2 * HD, qsl],
                    start=True,
                    stop=True,
                )
                irec[t] = {"stp": stp}

            def emit_act(t):
                pos, qc, kb = items[t]
                st_ = state[pos]
                ex = ex_pool.tile([P, 2, QC], BF16, name="ex")
                nc.scalar.activation(
                    ex[:],
                    irec[t]["stp"][:],
                    mybir.ActivationFunctionType.Exp,
                    bias=st_["mask"][:, kb:kb + 1],
                    scale=1.0 / np.sqrt(HD),
                )
                irec[t]["ex"] = ex

            def emit_ctx(t):
                pos, qc, kb = items[t]
                st_ = state[pos]
                von = st_["von"]
                if kb == 0:
                    ot0 = ps_ot.tile([P, QC], F32, name="ot0", tag="ot")
                    ot1 = ps_ot.tile([P, QC], F32, name="ot1", tag="ot")
                    otrec[(pos, qc)] = (ot0, ot1)
                ot0, ot1 = otrec[(pos, qc)]
                ex = irec[t]["ex"]
                nc.tensor.matmul(
                    ot0[0:HD + 1, :],
                    lhsT=von[:, kb, 0:HD + 1],
                    rhs=ex[:, 0, :],
                    start=(kb == 0),
                    stop=(kb == NKB - 1),
                )
                nc.tensor.matmul(
                    ot1[0:HD + 1, :],
                    lhsT=von[:, kb, HD + 1:2 * HD + 2],
                    rhs=ex[:, 1, :],
                    start=(kb == 0),
                    stop=(kb == NKB - 1),
                )
                del irec[t]
                if kb == NKB - 1:
                    # evacuate [d, q] context + l rows to sbuf, then HBM;
                    # the host divides by l and transposes.
                    qsl = slice(qc * QC, (qc + 1) * QC)
                    osb = osb_pool.tile([HD + 1, 2, QC], F32, name="osb")
                    nc.vector.tensor_copy(osb[:, 0, :], ot0[0:HD + 1, :])
                    nc.vector.tensor_copy(osb[:, 1, :], ot1[0:HD + 1, :])
                    nc.sync.dma_start(
                        out[seq[pos]].rearrange("h d s -> d h s")[:, :, qsl],
                        osb[:],
                    )
                    del otrec[(pos, qc)]

            emit_scores(0)
            for t in range(T):
                pos, qc, kb = items[t]
                if qc == 0 and kb == 0:
                    if pos + 1 < len(seq):
                        state[pos + 1] = start_b(seq[pos + 1])
                    state.pop(pos - 2, None)
                # PE slot order: row-tiled scores pair first (keeps the
                # ScalarE stream decoupled), projection fill, then the ctx
                # pair last so its von weight-load hides behind the
                # projections' full-mode matmuls.
                if t + 1 < T:
                    emit_scores(t + 1)
                emit_act(t)
                if own_pending:
                    for u in own_pending[:4]:
                        emit_unit(state[0], u)
                    del own_pending[:4]
                elif pos + 1 < len(seq):
                    units = state[pos + 1]["units"]
                    it = qc * NKB + kb
                    target = min(
                        len(units),
                        it * len(units) // (NQC * NKB - 16) + 1,
                    )
                    cur = inj.get(pos + 1, 0)
                    if cur < target:
                        emit_unit(state[pos + 1], units[cur])
                        inj[pos + 1] = cur + 1
                if t > 0:
                    emit_ctx(t - 1)
            emit_ctx(T - 1)
    _split_multi_waits(nc)
    return nc


def host_prep(hidden_states, attention_mask, Wq, bq, Wk, bk, Wv, bv):
    xt_np = np.ascontiguousarray(
        np.asarray(hidden_states).transpose(0, 2, 1)
    ).astype(ml_dtypes.bfloat16)
    mask_np = np.ascontiguousarray(
        np.asarray(attention_mask).reshape(B, S)
    ).astype(np.float32)
    def wprep(W, dsl):
        # [H, DH] slice -> transposed [H, DH] -> [p, hc*d] so the device
        # DMA is contiguous per partition
        wt = np.asarray(W)[dsl, :].T.astype(ml_dtypes.bfloat16)   # [H, DH]
        return np.ascontiguousarray(
            wt.reshape(NHC, P, DH).transpose(1, 0, 2).reshape(P, NHC * DH)
        )

    in_maps = []
    for c in range(N_CORES):
        dsl = slice(c * DH, (c + 1) * DH)
        in_maps.append(
            {
                "xt": xt_np,
                "wqt": wprep(Wq, dsl),
                "wkt": wprep(Wk, dsl),
                "wvt": wprep(Wv, dsl),
                "bqv": np.ascontiguousarray(np.asarray(bq)[dsl]).astype(np.float32),
                "bkv": np.ascontiguousarray(np.asarray(bk)[dsl]).astype(np.float32),
                "mask": mask_np,
            }
        )
    return in_maps


def gather(results, bv):
    out = np.empty((B, S, H), np.float32)
    for c in range(N_CORES):
        r = results[c]["out"]  # [B, 2, HD+1, S]
        ctx = r[:, :, 0:HD, :] / r[:, :, HD:HD + 1, :]   # normalize by l
        # [B, 2, HD, S] -> [B, S, 2*HD]
        out[:, :, c * DH:(c + 1) * DH] = (
            ctx.transpose(0, 3, 1, 2).reshape(B, S, DH)
        )
    # bv folded on the host: softmax rows sum to 1, so ctx(V+bv)=ctx(V)+bv
    out += np.asarray(bv).astype(np.float32)[None, None, :]
    return out


def make_runner(nc, in_maps):
    """Build a reusable jitted 8-core runner for `nc` (mirrors
    bass2jax.run_bass_via_pjrt's multi-core path, but keeps the jitted
    callable so repeated executions don't re-lower)."""
    import jax
    from jax.sharding import Mesh, NamedSharding, PartitionSpec
    from jax.experimental.shard_map import shard_map
    from concourse import bass2jax

    bass2jax.install_neuronx_cc_hook()
    partition_name = nc.partition_id_tensor.name if nc.partition_id_tensor else None
    in_names, out_names, out_avals, zero_outs = [], [], [], []
    for alloc in nc.m.functions[0].allocations:
        if not isinstance(alloc, mybir.MemoryLocationSet):
            continue
        name = alloc.memorylocations[0].name
        if alloc.kind == "ExternalInput":
            if name != partition_name:
                in_names.append(name)
        elif alloc.kind == "ExternalOutput":
            out_names.append(name)
            shape = tuple(alloc.tensor_shape)
            dtype = mybir.dt.np(alloc.dtype)
            out_avals.append(jax.core.ShapedArray(shape, dtype))
            zero_outs.append(np.zeros(shape, dtype))
    n_params = len(in_names)
    n_outs = len(out_avals)
    all_in = list(in_names) + list(out_names)
    if partition_name is not None:
        all_in.append(partition_name)

    def _body(*args):
        operands = list(args)
        if partition_name is not None:
            operands.append(bass2jax.partition_id_tensor())
        outs = bass2jax._bass_exec_p.bind(
            *operands,
            out_avals=tuple(out_avals),
            in_names=tuple(all_in),
            out_names=tuple(out_names),
            lowering_input_output_aliases=(),
            sim_require_finite=True,
            sim_require_nnan=True,
            nc=nc,
        )
        return tuple(outs)

    devices = jax.devices()[:N_CORES]
    mesh = Mesh(np.asarray(devices), ("core",))
    sharded = jax.jit(
        shard_map(
            _body,
            mesh=mesh,
            in_specs=(PartitionSpec("core"),) * (n_params + n_outs),
            out_specs=(PartitionSpec("core"),) * n_outs,
            check_rep=False,
        ),
        keep_unused=True,
    )
    per_core = [[np.asarray(m[name]) for name in in_names[:n_params]] for m in in_maps]
    concat_in = [
        np.concatenate([per_core[c][i] for c in range(N_CORES)], axis=0)
        for i in range(n_params)
    ]
    concat_zeros = [
        np.zeros((N_CORES * z.shape[0], *z.shape[1:]), z.dtype) for z in zero_outs
    ]
    sh = NamedSharding(mesh, PartitionSpec("core"))
    args_dev = [jax.device_put(a, sh) for a in concat_in] + [
        jax.device_put(a, sh) for a in concat_zeros
    ]

    def run():
        import jax as _jax

        outs = sharded(*args_dev)
        _jax.block_until_ready(outs)
        return [
            {
                name: np.asarray(outs[i]).reshape(N_CORES, *out_avals[i].shape)[c]
                for i, name in enumerate(out_names)
            }
            for c in range(N_CORES)
        ]

    def run_nofetch():
        import jax as _jax

        outs = sharded(*args_dev)
        _jax.block_until_ready(outs)

    run.nofetch = run_nofetch
    return run


def kernel(hidden_states, attention_mask, Wq, bq, Wk, bk, Wv, bv):
    in_maps = host_prep(hidden_states, attention_mask, Wq, bq, Wk, bk, Wv, bv)
    nc = build_bass()
    res = bass_utils.run_bass_kernel_spmd(nc, in_maps, core_ids=list(range(N_CORES)))
    return gather(res.results, bv)
